# revision 1
# baseline (speedup 1.0000x reference)
"""DeeperGCN (GENConv x4) forward on 8 Trainium2 NeuronCores — v2.

Differences vs v1 (see kernel_v1.py):
  - edge slots are PACKED per (window-group, chunk) section and padded with
    trailing -1 indices, which the dma_gather Q7 ucode trims — descriptor
    generation (the GpSimd bottleneck) now costs ~#real-edges, not capacity.
  - gather tables (t_full), messages, S one-hots, and MLP weights are bf16;
    AllGather volume halves.
  - h (residual) and t (conv input) live in SBUF for the whole program; only
    one big 3.2MB DMA per layer stages t to DRAM for the AllGather.
  - scatter matmuls follow a data-driven static schedule: each (window,
    chunk) contributes 1-2 S-matmuls whose tile positions are the union over
    the 8 cores (per-core S data zeroes the slots outside that core's range).
"""
import numpy as np

H = 128
L = 4
EPS_MSG = 1e-7
EPS_LN = 1e-5


class CFG:
    def __init__(self, n_nodes=100000, n_graphs=512, n_cores=8, win=128,
                 gw=4, nchunk=4):
        self.N = n_nodes
        self.G = n_graphs
        self.NC = n_cores
        self.SH = n_nodes // n_cores
        self.WIN = win
        self.SHP = ((self.SH + win - 1) // win) * win
        self.NW = self.SHP // win
        self.NCHUNK = nchunk
        assert (self.NC * self.SHP) % nchunk == 0
        self.CH = self.NC * self.SHP // nchunk
        assert self.CH <= 32767, "int16 gather index limit"
        self.GW = gw
        self.NGRP = (self.NW + gw - 1) // gw

    def grp_windows(self, g):
        return min(self.GW, self.NW - g * self.GW)


class Plan:
    """Static (core-independent) packing plan, derived from the max edge
    counts over all cores. Baked into the program; cache key must include
    the geometry tuple."""

    def __init__(self, cfg, counts):
        # counts: [NC, NGRP, NCHUNK] real edge counts per section
        c = cfg
        self.T = np.maximum(1, (counts.max(axis=0) + 127) // 128)  # tiles/section
        self.sec_tile_base = np.zeros((c.NGRP, c.NCHUNK), np.int64)
        t = 0
        for g in range(c.NGRP):
            for q in range(c.NCHUNK):
                self.sec_tile_base[g, q] = t
                t += self.T[g, q]
        self.NTILES = int(t)
        self.NSLOT = self.NTILES * 128
        self.key = (c.N, c.G, c.NC, c.GW, c.NCHUNK) + tuple(self.T.reshape(-1))


def bucket_core(cfg, core, src, dst, attr):
    """Per-core edges bucketed by (group, chunk), sorted by (window, crow)."""
    c = cfg
    sel = (dst // c.SH) == core
    s, d, a = src[sel], dst[sel], attr[sel]
    local = d - core * c.SH
    win = local // c.WIN
    dst_rel = local % c.WIN
    # table rows are partition-major: row = core*SHP + p*NW + w (so the
    # per-layer t_sb -> t_stage staging DMA is contiguous per partition)
    s_loc = s % c.SH
    pad_row = (s // c.SH) * c.SHP + (s_loc % c.WIN) * c.NW + s_loc // c.WIN
    chunk = pad_row // c.CH
    crow = pad_row % c.CH
    grp = win // c.GW
    order = np.lexsort((crow, win, chunk, grp))
    return (grp[order], chunk[order], win[order], dst_rel[order],
            crow[order], a[order])


def build_plan(cfg, bucketed):
    c = cfg
    counts = np.zeros((c.NC, c.NGRP, c.NCHUNK), np.int64)
    for core in range(c.NC):
        g_, q_, _, _, _, _ = bucketed[core]
        np.add.at(counts, (core, g_, q_), 1)
    return Plan(cfg, counts), counts


def build_smat_schedule(cfg, plan, bucketed, counts):
    """Static scatter schedule: per (g, w) a list of (q, tile_in_section,
    smat_slot). smat data is per-core. Returns (sched, NSMAT, grp_smat_base).
    """
    c = cfg
    # per (core, g, q, w): slot range within section
    win_lo = np.zeros((c.NC, c.NGRP, c.NCHUNK, c.GW), np.int64)
    win_hi = np.zeros((c.NC, c.NGRP, c.NCHUNK, c.GW), np.int64)
    for core in range(c.NC):
        g_, q_, w_, _, _, _ = bucketed[core]
        wg_ = w_ - g_ * c.GW
        # edges sorted by (g, q, w): cumulative offsets
        key = (g_ * c.NCHUNK + q_) * c.GW + wg_
        nkey = c.NGRP * c.NCHUNK * c.GW
        bc = np.bincount(key, minlength=nkey).reshape(c.NGRP, c.NCHUNK, c.GW)
        hi = np.cumsum(bc, axis=2)
        win_hi[core] = hi
        win_lo[core] = hi - bc
    sched = {}   # (g, w) -> list of (q, tile, smat_slot)
    nsmat = 0
    grp_smat_base = []
    for g in range(c.NGRP):
        grp_smat_base.append(nsmat)
        for wg in range(c.grp_windows(g)):
            lst = []
            for q in range(c.NCHUNK):
                lo = win_lo[:, g, q, wg].min() // 128
                hi_raw = win_hi[:, g, q, wg]
                # windows with zero edges on every core: skip
                if (hi_raw == win_lo[:, g, q, wg]).all():
                    continue
                hi = (hi_raw.max() - 1) // 128
                for t in range(lo, hi + 1):
                    lst.append((q, t, nsmat))
                    nsmat += 1
            sched[(g, wg)] = lst
    return sched, nsmat, grp_smat_base


def prep_core(cfg, plan, sched, core, bucketed, batch, Etab):
    c, p = cfg, plan
    g_, q_, w_, rel_, cr_, a_ = bucketed[core]
    nkey = c.NGRP * c.NCHUNK
    key = g_ * c.NCHUNK + q_
    bc = np.bincount(key, minlength=nkey)
    sec_off = np.concatenate([[0], np.cumsum(bc)])[:-1]

    slot_src = np.full(p.NSLOT, -1, np.int16)
    slot_rel = np.full(p.NSLOT, -1, np.int32)
    slot_attr = np.full(p.NSLOT, -1, np.int32)
    slot_w = np.full(p.NSLOT, -1, np.int32)
    # fill sections (edges already in (g, q, w, crow) order)
    pos = 0
    for g in range(c.NGRP):
        for q in range(c.NCHUNK):
            cnt = bc[g * c.NCHUNK + q]
            base = p.sec_tile_base[g, q] * 128
            sl = slice(pos, pos + cnt)
            slot_src[base:base + cnt] = cr_[sl]
            slot_rel[base:base + cnt] = rel_[sl]
            slot_attr[base:base + cnt] = a_[sl]
            slot_w[base:base + cnt] = w_[sl] - g * c.GW
            pos += cnt
    assert pos == len(g_)

    # idx buffers wrapped in 16 partitions, one call per (g, q)
    cols = []
    for g in range(c.NGRP):
        for q in range(c.NCHUNK):
            nidx = int(p.T[g, q]) * 128
            base = p.sec_tile_base[g, q] * 128
            lst = slot_src[base:base + nidx]
            arr = np.empty((128, nidx // 16), np.int16)
            cidx = np.arange(nidx // 16) * 16
            for pp in range(128):
                arr[pp, :] = lst[cidx + (pp % 16)]
            cols.append(arr)
    idx_buf = np.ascontiguousarray(np.concatenate(cols, axis=1))

    # S data per smat slot: [NSMAT, 128, WIN]
    nsmat = max(s for lst in sched.values() for (_, _, s) in lst) + 1
    S = np.zeros((nsmat, 128, c.WIN), np.float32)
    rel2 = slot_rel.reshape(p.NTILES, 128)
    w2 = slot_w.reshape(p.NTILES, 128)
    for (g, wg), lst in sched.items():
        for (q, t, sm) in lst:
            ti = p.sec_tile_base[g, q] + t
            mask = (w2[ti] == wg) & (rel2[ti] >= 0)
            rows = np.where(mask)[0]
            S[sm, rows, rel2[ti][rows]] = 1.0

    ap = slot_attr.reshape(p.NTILES, 128)
    E_pre = np.where(ap[:, :, None] >= 0,
                     Etab[np.clip(ap, 0, Etab.shape[0] - 1)], 0.0)

    # eps*deg and batch-relative per window
    deg = np.zeros(c.SHP, np.float32)
    sel_rel = slot_rel >= 0
    # recompute local dst index per real slot
    # window-of-slot: grp*GW + slot_w ; local = win*128 + rel
    tile_of_slot = np.arange(p.NSLOT) // 128
    grp_of_tile = np.zeros(p.NTILES, np.int64)
    for g in range(c.NGRP):
        for q in range(c.NCHUNK):
            b = p.sec_tile_base[g, q]
            grp_of_tile[b:b + p.T[g, q]] = g
    win_of_slot = grp_of_tile[tile_of_slot] * c.GW + slot_w
    loc = win_of_slot[sel_rel] * 128 + slot_rel[sel_rel]
    np.add.at(deg, loc, 1.0)
    eps_pm = np.ascontiguousarray((EPS_MSG * deg).reshape(c.NW, 128).T)

    b = batch[core * c.SH:(core + 1) * c.SH]
    g0 = int(b[0])
    batch_rel = np.full(c.SHP, -1.0, np.float32)
    batch_rel[:c.SH] = (b - g0).astype(np.float32)
    assert batch_rel.max() < 128
    batch_pm = np.ascontiguousarray(batch_rel.reshape(c.NW, 128).T)

    cnts = bc.astype(np.int32)   # [NGRP*NCHUNK] real edges per call
    return dict(idx_buf=idx_buf, S=S, E=E_pre, eps_pm=eps_pm,
                batch_pm=batch_pm, g0=g0, cnts=cnts)


def build_program(cfg, plan, sched, nsmat, grp_smat_base, trivial,
                  scratch=16384, tab_bf16=True, mlp_bufs=4):
    import concourse.bass as bass
    import concourse.bacc as bacc
    import concourse.mybir as mybir
    import concourse.tile as tile
    from concourse.masks import make_identity

    c, p = cfg, plan
    f32 = mybir.dt.float32
    bf16 = mybir.dt.bfloat16
    i16 = mybir.dt.int16
    AF = mybir.ActivationFunctionType
    OP = mybir.AluOpType

    nc = bacc.Bacc("TRN2", target_bir_lowering=False, debug=False,
                   num_devices=c.NC, dynamic_dma_scratch_size=scratch)

    tdt = bf16 if tab_bf16 else f32
    Tmax = int(p.T.max())
    grp_tiles = [int(p.T[g].sum()) for g in range(c.NGRP)]
    GT = max(grp_tiles)
    grp_smat = [
        (grp_smat_base[g + 1] if g + 1 < c.NGRP else nsmat) - grp_smat_base[g]
        for g in range(c.NGRP)]
    GS = max(grp_smat)

    # ---- DRAM inputs ----
    xT = nc.dram_tensor("xT", [128, c.SHP], bf16, kind="ExternalInput")
    We_d = nc.dram_tensor("We", [128, H], bf16, kind="ExternalInput")
    W1_d = nc.dram_tensor("W1", [L, 128, 2 * H], bf16, kind="ExternalInput")
    W2_d = nc.dram_tensor("W2", [L, 2, 128, H], bf16, kind="ExternalInput")
    idx_d = nc.dram_tensor("idx", [128, p.NSLOT // 16], i16, kind="ExternalInput")
    S_d = nc.dram_tensor("S", [nsmat, 128, c.WIN], bf16, kind="ExternalInput")
    E_d = nc.dram_tensor("E", [p.NTILES, 128, H], bf16, kind="ExternalInput")
    eps_d = nc.dram_tensor("epsdeg", [128, c.NW], f32, kind="ExternalInput")
    bat_d = nc.dram_tensor("batchrel", [128, c.NW], f32, kind="ExternalInput")
    iota_d = nc.dram_tensor("iota", [128, 128], f32, kind="ExternalInput")
    ncalls = c.NGRP * c.NCHUNK
    cnt_d = nc.dram_tensor("cnt", [128, ncalls], mybir.dt.int32,
                           kind="ExternalInput")
    aff_d = None
    if not trivial:
        aff_d = {
            "gn": nc.dram_tensor("gn", [L, 128, H], f32, kind="ExternalInput"),
            "bn": nc.dram_tensor("bn", [L, 128, H], f32, kind="ExternalInput"),
            "g1": nc.dram_tensor("g1", [L, 128, 2 * H], f32, kind="ExternalInput"),
            "bb1": nc.dram_tensor("bb1", [L, 128, 2 * H], f32, kind="ExternalInput"),
            "b1": nc.dram_tensor("b1", [L, 128, 2 * H], f32, kind="ExternalInput"),
            "b2": nc.dram_tensor("b2", [L, 128, H], f32, kind="ExternalInput"),
            "be": nc.dram_tensor("be", [128, H], f32, kind="ExternalInput"),
        }
    out_d = nc.dram_tensor("partial", [128, H], f32, kind="ExternalOutput")

    with tile.TileContext(nc) as tc:
        with tc.tile_pool(name="const", bufs=1) as cpool, \
             tc.tile_pool(name="msg", bufs=4) as msgpool, \
             tc.tile_pool(name="emb", bufs=2) as epool, \
             tc.tile_pool(name="smat", bufs=2) as spool, \
             tc.tile_pool(name="mbf", bufs=3) as mbfpool, \
             tc.tile_pool(name="mlp", bufs=mlp_bufs) as mlppool, \
             tc.tile_pool(name="small", bufs=4) as smpool, \
             tc.tile_pool(name="psA", bufs=3, space="PSUM") as psA, \
             tc.tile_pool(name="psB", bufs=2, space="PSUM") as psB, \
             tc.tile_pool(name="psPool", bufs=1, space="PSUM") as psP, \
             tc.tile_pool(name="dram", bufs=1, space="DRAM") as dpool:

            # ---- persistent DRAM state ----
            t_stage = dpool.tile([c.SHP, H], tdt)
            t_fulls = []
            for l in range(L):
                tf = dpool.tile([c.NC * c.SHP, H], tdt, addr_space="Shared",
                                tag=f"t_full{l}")
                t_fulls.append(tf)

            # ---- resident constants / state ----
            identf = cpool.tile([128, 128], f32)
            make_identity(nc, identf[:])
            identb = cpool.tile([128, 128], bf16)
            make_identity(nc, identb[:])
            We_sb = cpool.tile([128, H], bf16)
            nc.sync.dma_start(We_sb[:], We_d[:])
            W1_sb = cpool.tile([128, L, 2 * H], bf16)
            nc.sync.dma_start(W1_sb[:], W1_d[:].rearrange("l k n -> k l n"))
            W2_sb = cpool.tile([128, L, 2, H], bf16)
            nc.sync.dma_start(W2_sb[:], W2_d[:].rearrange("l j k n -> k l j n"))
            idx_sb = cpool.tile([128, p.NSLOT // 16], i16)
            nc.sync.dma_start(idx_sb[:], idx_d[:])
            eps_sb = cpool.tile([128, c.NW], f32)
            nc.sync.dma_start(eps_sb[:], eps_d[:])
            bat_sb = cpool.tile([128, c.NW], f32)
            nc.sync.dma_start(bat_sb[:], bat_d[:])
            iota_sb = cpool.tile([128, 128], f32)
            nc.sync.dma_start(iota_sb[:], iota_d[:])
            cnt_sb = cpool.tile([128, ncalls], mybir.dt.int32)
            nc.sync.dma_start(cnt_sb[:], cnt_d[:])
            cnt_reg = nc.gpsimd.alloc_register("cnt_reg")
            epsln_sb = cpool.tile([128, 1], f32)
            nc.vector.memset(epsln_sb[:], EPS_LN)
            t_sb = cpool.tile([128, c.NW, H], tdt)
            h_sb = cpool.tile([128, c.NW, H], f32)
            aff_sb = {}
            if not trivial:
                for k, dd in aff_d.items():
                    if k == "be":
                        t_ = cpool.tile([128, H], f32)
                        nc.sync.dma_start(t_[:], dd[:])
                    else:
                        t_ = cpool.tile([128, L, dd.shape[-1]], f32)
                        nc.sync.dma_start(t_[:], dd[:].rearrange("l p n -> p l n"))
                    aff_sb[k] = t_

            def ln_relu_fused(dst, src_ap, gname, bname, lidx, relu, width):
                st = smpool.tile([128, 6], f32, tag="st")
                nc.vector.bn_stats(st[:], src_ap)
                mv = smpool.tile([128, 2], f32, tag="mv")
                nc.vector.bn_aggr(mv[:], st[:])
                std = smpool.tile([128, 1], f32, tag="std")
                nc.scalar.activation(std[:], mv[:, 1:2], AF.Sqrt, bias=epsln_sb[:, 0:1])
                rstd = smpool.tile([128, 1], f32, tag="rstd")
                nc.vector.reciprocal(rstd[:], std[:])
                nb = smpool.tile([128, 1], f32, tag="nb")
                nc.vector.tensor_scalar(nb[:], mv[:, 0:1], rstd[:, 0:1], -1.0,
                                        OP.mult, OP.mult)
                if trivial:
                    nc.scalar.activation(dst, src_ap,
                                         AF.Relu if relu else AF.Identity,
                                         bias=nb[:, 0:1], scale=rstd[:, 0:1])
                else:
                    z = mlppool.tile([128, width], f32, tag=f"lnz{width}")
                    nc.scalar.activation(z[:], src_ap, AF.Identity,
                                         bias=nb[:, 0:1], scale=rstd[:, 0:1])
                    g_ap = aff_sb[gname][:, lidx, :]
                    b_ap = aff_sb[bname][:, lidx, :]
                    nc.vector.tensor_tensor(z[:], z[:], g_ap, op=OP.mult)
                    if relu:
                        nc.vector.tensor_tensor(z[:], z[:], b_ap, op=OP.add)
                        nc.scalar.activation(dst, z[:], AF.Relu)
                    else:
                        nc.vector.tensor_tensor(dst, z[:], b_ap, op=OP.add)

            # ================= encoder =================
            for w in range(c.NW):
                xt_t = mlppool.tile([128, 128], bf16, tag="xt_enc")
                nc.sync.dma_start(xt_t[:], xT[:, w * 128:(w + 1) * 128])
                h0_ps = psB.tile([128, H], f32, tag="tr")
                nc.tensor.matmul(h0_ps[:], xt_t[:], We_sb[:], start=True, stop=True)
                if trivial:
                    nc.vector.tensor_copy(h_sb[:, w, :], h0_ps[:])
                else:
                    nc.vector.tensor_tensor(h_sb[:, w, :], h0_ps[:],
                                            aff_sb["be"][:], op=OP.add)
                nc.scalar.activation(t_sb[:, w, :], h_sb[:, w, :], AF.Identity)

            rg = [list(range(c.NC))]

            def stage_full():
                nc.sync.dma_start(
                    t_stage[:].rearrange("(pp w) h -> pp (w h)", pp=128),
                    t_sb[:].rearrange("pp w h -> pp (w h)"))

            def allgather(l):
                nc.gpsimd.collective_compute(
                    "AllGather", OP.bypass, replica_groups=rg,
                    ins=[t_stage[:]], outs=[t_fulls[l][:]])

            stage_full()
            allgather(0)

            # memset message pool buffers once (stale-SBUF guard: padding
            # slots are never DMA'd; S zeros them, but NaN*0 would poison PE)
            for par in range(4):
                for q in range(c.NCHUNK):
                    mt = msgpool.tile([128, Tmax, 128], tdt, tag=f"msg{q}")
                    nc.vector.memset(mt[:].rearrange("pp t n -> pp (t n)"), 0.0)

            # ================= conv layers =================
            pool_ps = None
            for l in range(L):
                for g in range(c.NGRP):
                    gw = c.grp_windows(g)
                    gtb = int(p.sec_tile_base[g, 0])
                    ntile_g = grp_tiles[g]
                    smb = grp_smat_base[g]
                    nsm_g = grp_smat[g]
                    s_t = spool.tile([128, GS, c.WIN], bf16, tag="s")
                    nc.sync.dma_start(
                        s_t[:, 0:nsm_g, :],
                        S_d[smb:smb + nsm_g, :, :].rearrange("t pp n -> pp t n"))
                    e_t = epool.tile([128, GT, H], bf16, tag="e")
                    nc.sync.dma_start(
                        e_t[:, 0:ntile_g, :],
                        E_d[gtb:gtb + ntile_g, :, :].rearrange("t pp n -> pp t n"))
                    m_bf = mbfpool.tile([128, GT, 128], bf16, tag="mbf")
                    for q in range(c.NCHUNK):
                        Tq = int(p.T[g, q])
                        nidx = Tq * 128
                        toff = int(p.sec_tile_base[g, q]) - gtb
                        msg = msgpool.tile([128, Tmax, 128], tdt, tag=f"msg{q}")
                        colbase = (gtb + toff) * 8
                        ci = g * c.NCHUNK + q
                        nc.gpsimd.reg_load(cnt_reg, cnt_sb[0:1, ci:ci + 1])
                        nc.gpsimd.dma_gather(
                            msg[:, 0:Tq, :],
                            t_fulls[l][q * c.CH:(q + 1) * c.CH, :],
                            idx_sb[:, colbase:colbase + nidx // 16],
                            nidx, cnt_reg, elem_size=H, elem_step=H,
                            single_packet=False)
                        msl = msg[:, 0:Tq, :].rearrange("pp t n -> pp (t n)")
                        nc.vector.tensor_tensor(
                            msl, msl,
                            e_t[:, toff:toff + Tq, :].rearrange("pp t n -> pp (t n)"),
                            op=OP.add)
                        nc.vector.tensor_scalar(
                            m_bf[:, toff:toff + Tq, :].rearrange("pp t n -> pp (t n)"),
                            msl, 0.0, EPS_MSG, OP.max, OP.add)
                    for wg in range(gw):
                        w = g * c.GW + wg
                        lst = sched[(g, wg)]
                        agg_ps = psA.tile([128, H], f32, tag="agg")
                        if not lst:
                            nc.vector.memset(agg_ps[:], 0.0)
                        else:
                            for j, (q, t, sm) in enumerate(lst):
                                ti = int(p.sec_tile_base[g, q]) - gtb + t
                                nc.tensor.matmul(
                                    agg_ps[:], s_t[:, sm - smb, :],
                                    m_bf[:, ti, :],
                                    start=(j == 0), stop=(j == len(lst) - 1))
                        # ---- window MLP ----
                        X = mlppool.tile([128, H], f32, tag="X")
                        nc.vector.tensor_tensor(X[:], agg_ps[:], t_sb[:, w, :],
                                                op=OP.add)
                        xt_ps = psB.tile([128, 128], f32, tag="tr")
                        nc.tensor.transpose(xt_ps[:], X[:], identf[:])
                        XT = mlppool.tile([128, 128], bf16, tag="XT")
                        nc.scalar.activation(XT[:], xt_ps[:], AF.Identity)
                        y1_ps = psB.tile([128, 2 * H], f32, tag="y")
                        nc.tensor.matmul(y1_ps[:], XT[:], W1_sb[:, l, :],
                                         start=True, stop=True)
                        y1 = mlppool.tile([128, 2 * H], f32, tag="y1sb")
                        if not trivial:
                            nc.vector.tensor_tensor(y1[:], y1_ps[:],
                                                    aff_sb["b1"][:, l, :], op=OP.add)
                        else:
                            nc.scalar.activation(y1[:], y1_ps[:], AF.Identity)
                        z2 = mlppool.tile([128, 2 * H], f32, tag="z2")
                        ln_relu_fused(z2[:], y1[:], "g1", "bb1", l,
                                      relu=True, width=2 * H)
                        z2t = mlppool.tile([128, 2, 128], bf16, tag="z2t")
                        for kk in range(2):
                            zt_ps = psB.tile([128, 128], f32, tag="tr")
                            nc.tensor.transpose(zt_ps[:], z2[:, kk * 128:(kk + 1) * 128],
                                                identf[:])
                            nc.scalar.activation(z2t[:, kk, :], zt_ps[:], AF.Identity)
                        y2_ps = psB.tile([128, H], f32, tag="y")
                        for kk in range(2):
                            nc.tensor.matmul(y2_ps[:], z2t[:, kk, :],
                                             W2_sb[:, l, kk, :],
                                             start=(kk == 0), stop=(kk == 1))
                        hn = mlppool.tile([128, H], f32, tag="hn")
                        if l > 0:
                            nc.vector.tensor_tensor(hn[:], y2_ps[:],
                                                    h_sb[:, w, :], op=OP.add)
                        else:
                            nc.vector.tensor_copy(hn[:], y2_ps[:])
                        if not trivial:
                            nc.vector.tensor_tensor(hn[:], hn[:],
                                                    aff_sb["b2"][:, l, :], op=OP.add)
                        if l < L - 1:
                            nc.scalar.activation(h_sb[:, w, :], hn[:], AF.Identity)
                            ln_relu_fused(t_sb[:, w, :], hn[:], "gn", "bn", l,
                                          relu=True, width=H)
                        else:
                            hf = mlppool.tile([128, H], bf16, tag="hf")
                            ln_relu_fused(hf[:], hn[:], "gn", "bn", l,
                                          relu=False, width=H)
                            Sg = mlppool.tile([128, 128], bf16, tag="Sg")
                            nc.vector.tensor_scalar(Sg[:], iota_sb[:],
                                                    bat_sb[:, w:w + 1], None,
                                                    OP.is_equal)
                            if pool_ps is None:
                                pool_ps = psP.tile([128, H], f32, tag="pool")
                            nc.tensor.matmul(pool_ps[:], Sg[:], hf[:],
                                             start=(w == 0), stop=(w == c.NW - 1),
                                             skip_group_check=True)
                if l < L - 1:
                    stage_full()
                    allgather(l + 1)
            psb = mlppool.tile([128, H], f32, tag="psb")
            nc.vector.tensor_copy(psb[:], pool_ps[:])
            nc.sync.dma_start(out_d[:], psb[:])

    nc.compile()
    return nc


def make_inputs(cfg, inp):
    c = cfg
    import ml_dtypes
    src = np.asarray(inp['edge_index'][0], np.int64)
    dst = np.asarray(inp['edge_index'][1], np.int64)
    attr = np.asarray(inp['edge_attr'], np.int64)
    batch = np.asarray(inp['batch'], np.int64)
    x = np.asarray(inp['x'], np.float32)
    Etab = np.asarray(inp['Etab'], np.float32)
    We = np.asarray(inp['We'], np.float32)
    W1 = np.asarray(inp['W1'], np.float32)
    W2 = np.asarray(inp['W2'], np.float32)

    trivial = (np.all(np.asarray(inp['be']) == 0) and np.all(np.asarray(inp['b1']) == 0)
               and np.all(np.asarray(inp['g1']) == 1) and np.all(np.asarray(inp['bb1']) == 0)
               and np.all(np.asarray(inp['b2']) == 0) and np.all(np.asarray(inp['gn']) == 1)
               and np.all(np.asarray(inp['bn']) == 0))

    bucketed = [bucket_core(c, core, src, dst, attr) for core in range(c.NC)]
    plan, counts = build_plan(c, bucketed)
    sched, nsmat, grp_smat_base = build_smat_schedule(c, plan, bucketed, counts)

    W2s = np.ascontiguousarray(W2.reshape(L, 2, 128, H))
    iota = np.tile(np.arange(128, dtype=np.float32)[None, :], (128, 1))
    bf = ml_dtypes.bfloat16
    in_maps, metas = [], []
    for core in range(c.NC):
        cd = prep_core(c, plan, sched, core, bucketed, batch, Etab)
        xs = x[core * c.SH:(core + 1) * c.SH]
        xTp = np.zeros((128, c.SHP), np.float32)
        xTp[:, :c.SH] = xs.T
        m = {
            'xT': xTp.astype(bf), 'We': We.astype(bf),
            'W1': W1.astype(bf), 'W2': W2s.astype(bf),
            'idx': cd['idx_buf'],
            'S': cd['S'].astype(bf),
            'E': cd['E'].astype(bf),
            'epsdeg': cd['eps_pm'], 'batchrel': cd['batch_pm'],
            'iota': iota,
            'cnt': np.tile(cd['cnts'][None, :], (128, 1)),
        }
        if not trivial:
            rep = lambda v, wdt: np.tile(np.asarray(v, np.float32)[:, None, :], (1, 128, 1))
            m['gn'] = rep(inp['gn'], H); m['bn'] = rep(inp['bn'], H)
            m['g1'] = rep(inp['g1'], 2 * H); m['bb1'] = rep(inp['bb1'], 2 * H)
            m['b1'] = rep(inp['b1'], 2 * H); m['b2'] = rep(inp['b2'], H)
            m['be'] = np.tile(np.asarray(inp['be'], np.float32)[None, :], (128, 1))
        in_maps.append(m)
        metas.append(cd)
    return in_maps, metas, trivial, plan, sched, nsmat, grp_smat_base


def postprocess(cfg, inp, results, metas):
    c = cfg
    batch = np.asarray(inp['batch'], np.int64)
    sums = np.zeros((c.G, H), np.float32)
    for core in range(c.NC):
        part = results[core]['partial']
        g0 = metas[core]['g0']
        b = batch[core * c.SH:(core + 1) * c.SH]
        gmax = int(b.max()) - g0
        sums[g0:g0 + gmax + 1] += part[:gmax + 1]
    cnt = np.bincount(batch, minlength=c.G).astype(np.float32)
    h_graph = sums / np.maximum(cnt, 1.0)[:, None]
    Wp = np.asarray(inp['Wp'], np.float32)
    bp = np.asarray(inp['bp'], np.float32)
    logits = h_graph @ Wp + bp
    return (1.0 / (1.0 + np.exp(-logits))).reshape(-1).astype(np.float32)


_CACHE = {}


def kernel(**inputs):
    from concourse.bass_utils import run_bass_kernel_spmd
    cfg = CFG()
    in_maps, metas, trivial, plan, sched, nsmat, gsb = make_inputs(cfg, inputs)
    key = ('prog', trivial, plan.key)
    if key not in _CACHE:
        _CACHE[key] = build_program(cfg, plan, sched, nsmat, gsb, trivial)
    nc = _CACHE[key]
    res = run_bass_kernel_spmd(nc, in_maps, core_ids=list(range(cfg.NC)))
    return postprocess(cfg, inputs, res.results, metas)



# revision 4
# speedup vs baseline: 1.5275x; 1.5275x over previous
"""DeeperGCN (GENConv x4) forward on 8 Trainium2 NeuronCores — v2.

Differences vs v1 (see kernel_v1.py):
  - edge slots are PACKED per (window-group, chunk) section and padded with
    trailing -1 indices, which the dma_gather Q7 ucode trims — descriptor
    generation (the GpSimd bottleneck) now costs ~#real-edges, not capacity.
  - gather tables (t_full), messages, S one-hots, and MLP weights are bf16;
    AllGather volume halves.
  - h (residual) and t (conv input) live in SBUF for the whole program; only
    one big 3.2MB DMA per layer stages t to DRAM for the AllGather.
  - scatter matmuls follow a data-driven static schedule: each (window,
    chunk) contributes 1-2 S-matmuls whose tile positions are the union over
    the 8 cores (per-core S data zeroes the slots outside that core's range).
"""
import numpy as np

H = 128
L = 4
EPS_MSG = 1e-7
EPS_LN = 1e-5


class CFG:
    def __init__(self, n_nodes=100000, n_graphs=512, n_cores=8, win=128,
                 gw=4, nchunk=4):
        self.N = n_nodes
        self.G = n_graphs
        self.NC = n_cores
        self.SH = n_nodes // n_cores
        self.WIN = win
        self.SHP = ((self.SH + win - 1) // win) * win
        self.NW = self.SHP // win
        self.NCHUNK = nchunk
        assert (self.NC * self.SHP) % nchunk == 0
        self.CH = self.NC * self.SHP // nchunk
        assert self.CH <= 32767, "int16 gather index limit"
        self.GW = gw
        self.NGRP = (self.NW + gw - 1) // gw

    def grp_windows(self, g):
        return min(self.GW, self.NW - g * self.GW)


class Plan:
    """Static (core-independent) packing plan, derived from the max edge
    counts over all cores. Baked into the program; cache key must include
    the geometry tuple."""

    def __init__(self, cfg, counts):
        # counts: [NC, NGRP, NCHUNK] real edge counts per section
        c = cfg
        self.T = np.maximum(1, (counts.max(axis=0) + 127) // 128)  # tiles/section
        self.sec_tile_base = np.zeros((c.NGRP, c.NCHUNK), np.int64)
        t = 0
        for g in range(c.NGRP):
            for q in range(c.NCHUNK):
                self.sec_tile_base[g, q] = t
                t += self.T[g, q]
        self.NTILES = int(t)
        self.NSLOT = self.NTILES * 128
        self.key = (c.N, c.G, c.NC, c.GW, c.NCHUNK) + tuple(self.T.reshape(-1))


def bucket_core(cfg, core, src, dst, attr):
    """Per-core edges bucketed by (group, chunk), sorted by (window, crow)."""
    c = cfg
    sel = (dst // c.SH) == core
    s, d, a = src[sel], dst[sel], attr[sel]
    local = d - core * c.SH
    win = local // c.WIN
    dst_rel = local % c.WIN
    # table rows are partition-major: row = core*SHP + p*NW + w (so the
    # per-layer t_sb -> t_stage staging DMA is contiguous per partition)
    s_loc = s % c.SH
    pad_row = (s // c.SH) * c.SHP + (s_loc % c.WIN) * c.NW + s_loc // c.WIN
    chunk = pad_row // c.CH
    crow = pad_row % c.CH
    grp = win // c.GW
    order = np.lexsort((crow, win, chunk, grp))
    return (grp[order], chunk[order], win[order], dst_rel[order],
            crow[order], a[order])


def build_plan(cfg, bucketed):
    c = cfg
    counts = np.zeros((c.NC, c.NGRP, c.NCHUNK), np.int64)
    for core in range(c.NC):
        g_, q_, _, _, _, _ = bucketed[core]
        np.add.at(counts, (core, g_, q_), 1)
    return Plan(cfg, counts), counts


def build_smat_schedule(cfg, plan, bucketed, counts):
    """Static scatter schedule: per (g, w) a list of (q, tile_in_section,
    smat_slot). smat data is per-core. Returns (sched, NSMAT, grp_smat_base).
    """
    c = cfg
    # per (core, g, q, w): slot range within section
    win_lo = np.zeros((c.NC, c.NGRP, c.NCHUNK, c.GW), np.int64)
    win_hi = np.zeros((c.NC, c.NGRP, c.NCHUNK, c.GW), np.int64)
    for core in range(c.NC):
        g_, q_, w_, _, _, _ = bucketed[core]
        wg_ = w_ - g_ * c.GW
        # edges sorted by (g, q, w): cumulative offsets
        key = (g_ * c.NCHUNK + q_) * c.GW + wg_
        nkey = c.NGRP * c.NCHUNK * c.GW
        bc = np.bincount(key, minlength=nkey).reshape(c.NGRP, c.NCHUNK, c.GW)
        hi = np.cumsum(bc, axis=2)
        win_hi[core] = hi
        win_lo[core] = hi - bc
    sched = {}   # (g, w) -> list of (q, tile, smat_slot)
    nsmat = 0
    grp_smat_base = []
    for g in range(c.NGRP):
        grp_smat_base.append(nsmat)
        for wg in range(c.grp_windows(g)):
            lst = []
            for q in range(c.NCHUNK):
                lo = win_lo[:, g, q, wg].min() // 128
                hi_raw = win_hi[:, g, q, wg]
                # windows with zero edges on every core: skip
                if (hi_raw == win_lo[:, g, q, wg]).all():
                    continue
                hi = (hi_raw.max() - 1) // 128
                for t in range(lo, hi + 1):
                    lst.append((q, t, nsmat))
                    nsmat += 1
            sched[(g, wg)] = lst
    return sched, nsmat, grp_smat_base


def prep_core(cfg, plan, sched, core, bucketed, batch, Etab):
    c, p = cfg, plan
    g_, q_, w_, rel_, cr_, a_ = bucketed[core]
    nkey = c.NGRP * c.NCHUNK
    key = g_ * c.NCHUNK + q_
    bc = np.bincount(key, minlength=nkey)
    sec_off = np.concatenate([[0], np.cumsum(bc)])[:-1]

    slot_src = np.full(p.NSLOT, -1, np.int16)
    slot_rel = np.full(p.NSLOT, -1, np.int32)
    slot_attr = np.full(p.NSLOT, -1, np.int32)
    slot_w = np.full(p.NSLOT, -1, np.int32)
    # fill sections (edges already in (g, q, w, crow) order)
    pos = 0
    for g in range(c.NGRP):
        for q in range(c.NCHUNK):
            cnt = bc[g * c.NCHUNK + q]
            base = p.sec_tile_base[g, q] * 128
            sl = slice(pos, pos + cnt)
            slot_src[base:base + cnt] = cr_[sl]
            slot_rel[base:base + cnt] = rel_[sl]
            slot_attr[base:base + cnt] = a_[sl]
            slot_w[base:base + cnt] = w_[sl] - g * c.GW
            pos += cnt
    assert pos == len(g_)

    # idx buffers wrapped in 16 partitions, one call per (g, q)
    cols = []
    for g in range(c.NGRP):
        for q in range(c.NCHUNK):
            nidx = int(p.T[g, q]) * 128
            base = p.sec_tile_base[g, q] * 128
            lst = slot_src[base:base + nidx]
            arr = np.empty((128, nidx // 16), np.int16)
            cidx = np.arange(nidx // 16) * 16
            for pp in range(128):
                arr[pp, :] = lst[cidx + (pp % 16)]
            cols.append(arr)
    idx_buf = np.ascontiguousarray(np.concatenate(cols, axis=1))

    # S data per smat slot: [NSMAT, 128, WIN]
    nsmat = max(s for lst in sched.values() for (_, _, s) in lst) + 1
    S = np.zeros((nsmat, 128, c.WIN), np.float32)
    rel2 = slot_rel.reshape(p.NTILES, 128)
    w2 = slot_w.reshape(p.NTILES, 128)
    for (g, wg), lst in sched.items():
        for (q, t, sm) in lst:
            ti = p.sec_tile_base[g, q] + t
            mask = (w2[ti] == wg) & (rel2[ti] >= 0)
            rows = np.where(mask)[0]
            S[sm, rows, rel2[ti][rows]] = 1.0

    ap = slot_attr.reshape(p.NTILES, 128)
    E_pre = np.where(ap[:, :, None] >= 0,
                     Etab[np.clip(ap, 0, Etab.shape[0] - 1)], 0.0)

    # eps*deg and batch-relative per window
    deg = np.zeros(c.SHP, np.float32)
    sel_rel = slot_rel >= 0
    # recompute local dst index per real slot
    # window-of-slot: grp*GW + slot_w ; local = win*128 + rel
    tile_of_slot = np.arange(p.NSLOT) // 128
    grp_of_tile = np.zeros(p.NTILES, np.int64)
    for g in range(c.NGRP):
        for q in range(c.NCHUNK):
            b = p.sec_tile_base[g, q]
            grp_of_tile[b:b + p.T[g, q]] = g
    win_of_slot = grp_of_tile[tile_of_slot] * c.GW + slot_w
    loc = win_of_slot[sel_rel] * 128 + slot_rel[sel_rel]
    np.add.at(deg, loc, 1.0)
    eps_pm = np.ascontiguousarray((EPS_MSG * deg).reshape(c.NW, 128).T)

    b = batch[core * c.SH:(core + 1) * c.SH]
    g0 = int(b[0])
    batch_rel = np.full(c.SHP, -1.0, np.float32)
    batch_rel[:c.SH] = (b - g0).astype(np.float32)
    assert batch_rel.max() < 128
    batch_pm = np.ascontiguousarray(batch_rel.reshape(c.NW, 128).T)

    cnts = bc.astype(np.int32)   # [NGRP*NCHUNK] real edges per call
    return dict(idx_buf=idx_buf, S=S, E=E_pre, eps_pm=eps_pm,
                batch_pm=batch_pm, g0=g0, cnts=cnts)


def build_program(cfg, plan, sched, nsmat, grp_smat_base, trivial,
                  scratch=16384, tab_bf16=True, mlp_bufs=4):
    import concourse.bass as bass
    import concourse.bacc as bacc
    import concourse.mybir as mybir
    import concourse.tile as tile
    from concourse.masks import make_identity

    c, p = cfg, plan
    f32 = mybir.dt.float32
    bf16 = mybir.dt.bfloat16
    i16 = mybir.dt.int16
    AF = mybir.ActivationFunctionType
    OP = mybir.AluOpType

    nc = bacc.Bacc("TRN2", target_bir_lowering=False, debug=False,
                   num_devices=c.NC, dynamic_dma_scratch_size=scratch,
                   num_swdge_queues=min(4, c.NCHUNK))

    tdt = bf16 if tab_bf16 else f32
    Tmax = int(p.T.max())
    grp_tiles = [int(p.T[g].sum()) for g in range(c.NGRP)]
    GT = max(grp_tiles)
    grp_smat = [
        (grp_smat_base[g + 1] if g + 1 < c.NGRP else nsmat) - grp_smat_base[g]
        for g in range(c.NGRP)]
    GS = max(grp_smat)

    # ---- DRAM inputs ----
    xT = nc.dram_tensor("xT", [128, c.SHP], bf16, kind="ExternalInput")
    We_d = nc.dram_tensor("We", [128, H], bf16, kind="ExternalInput")
    W1_d = nc.dram_tensor("W1", [L, 128, 2 * H], bf16, kind="ExternalInput")
    W2_d = nc.dram_tensor("W2", [L, 2, 128, H], bf16, kind="ExternalInput")
    idx_d = nc.dram_tensor("idx", [128, p.NSLOT // 16], i16, kind="ExternalInput")
    S_d = nc.dram_tensor("S", [nsmat, 128, c.WIN], bf16, kind="ExternalInput")
    E_d = nc.dram_tensor("E", [p.NTILES, 128, H], bf16, kind="ExternalInput")
    eps_d = nc.dram_tensor("epsdeg", [128, c.NW], f32, kind="ExternalInput")
    bat_d = nc.dram_tensor("batchrel", [128, c.NW], f32, kind="ExternalInput")
    iota_d = nc.dram_tensor("iota", [128, 128], f32, kind="ExternalInput")
    ncalls = c.NGRP * c.NCHUNK
    cnt_d = nc.dram_tensor("cnt", [128, ncalls], mybir.dt.int32,
                           kind="ExternalInput")
    aff_d = None
    if not trivial:
        aff_d = {
            "gn": nc.dram_tensor("gn", [L, 128, H], f32, kind="ExternalInput"),
            "bn": nc.dram_tensor("bn", [L, 128, H], f32, kind="ExternalInput"),
            "g1": nc.dram_tensor("g1", [L, 128, 2 * H], f32, kind="ExternalInput"),
            "bb1": nc.dram_tensor("bb1", [L, 128, 2 * H], f32, kind="ExternalInput"),
            "b1": nc.dram_tensor("b1", [L, 128, 2 * H], f32, kind="ExternalInput"),
            "b2": nc.dram_tensor("b2", [L, 128, H], f32, kind="ExternalInput"),
            "be": nc.dram_tensor("be", [128, H], f32, kind="ExternalInput"),
        }
    out_d = nc.dram_tensor("partial", [128, H], f32, kind="ExternalOutput")

    with tile.TileContext(nc) as tc:
        with tc.tile_pool(name="const", bufs=1) as cpool, \
             tc.tile_pool(name="msg", bufs=4) as msgpool, \
             tc.tile_pool(name="emb", bufs=2) as epool, \
             tc.tile_pool(name="smat", bufs=2) as spool, \
             tc.tile_pool(name="mbf", bufs=3) as mbfpool, \
             tc.tile_pool(name="mlp", bufs=mlp_bufs) as mlppool, \
             tc.tile_pool(name="small", bufs=4) as smpool, \
             tc.tile_pool(name="psA", bufs=3, space="PSUM") as psA, \
             tc.tile_pool(name="psB", bufs=2, space="PSUM") as psB, \
             tc.tile_pool(name="psPool", bufs=1, space="PSUM") as psP, \
             tc.tile_pool(name="dram", bufs=1, space="DRAM") as dpool:

            # ---- persistent DRAM state ----
            t_stage = dpool.tile([c.SHP, H], tdt)
            t_fulls = []
            for l in range(L):
                tf = dpool.tile([c.NC * c.SHP, H], tdt, addr_space="Shared",
                                tag=f"t_full{l}")
                t_fulls.append(tf)

            # ---- resident constants / state ----
            identf = cpool.tile([128, 128], f32)
            make_identity(nc, identf[:])
            identb = cpool.tile([128, 128], bf16)
            make_identity(nc, identb[:])
            We_sb = cpool.tile([128, H], bf16)
            nc.sync.dma_start(We_sb[:], We_d[:])
            W1_sb = cpool.tile([128, L, 2 * H], bf16)
            nc.sync.dma_start(W1_sb[:], W1_d[:].rearrange("l k n -> k l n"))
            W2_sb = cpool.tile([128, L, 2, H], bf16)
            nc.sync.dma_start(W2_sb[:], W2_d[:].rearrange("l j k n -> k l j n"))
            idx_sb = cpool.tile([128, p.NSLOT // 16], i16)
            nc.sync.dma_start(idx_sb[:], idx_d[:])
            eps_sb = cpool.tile([128, c.NW], f32)
            nc.sync.dma_start(eps_sb[:], eps_d[:])
            bat_sb = cpool.tile([128, c.NW], f32)
            nc.sync.dma_start(bat_sb[:], bat_d[:])
            iota_sb = cpool.tile([128, 128], f32)
            nc.sync.dma_start(iota_sb[:], iota_d[:])
            cnt_sb = cpool.tile([128, ncalls], mybir.dt.int32)
            nc.sync.dma_start(cnt_sb[:], cnt_d[:])
            nq = min(4, c.NCHUNK)
            cnt_regs = [nc.gpsimd.alloc_register(f"cnt_reg{q}")
                        for q in range(nq)]
            epsln_sb = cpool.tile([128, 1], f32)
            nc.vector.memset(epsln_sb[:], EPS_LN)
            t_sb = cpool.tile([128, c.NW, H], tdt)
            h_sb = cpool.tile([128, c.NW, H], f32)
            aff_sb = {}
            if not trivial:
                for k, dd in aff_d.items():
                    if k == "be":
                        t_ = cpool.tile([128, H], f32)
                        nc.sync.dma_start(t_[:], dd[:])
                    else:
                        t_ = cpool.tile([128, L, dd.shape[-1]], f32)
                        nc.sync.dma_start(t_[:], dd[:].rearrange("l p n -> p l n"))
                    aff_sb[k] = t_

            def ln_relu_fused(dst, src_ap, gname, bname, lidx, relu, width):
                st = smpool.tile([128, 6], f32, tag="st")
                nc.vector.bn_stats(st[:], src_ap)
                mv = smpool.tile([128, 2], f32, tag="mv")
                nc.vector.bn_aggr(mv[:], st[:])
                std = smpool.tile([128, 1], f32, tag="std")
                nc.scalar.activation(std[:], mv[:, 1:2], AF.Sqrt, bias=epsln_sb[:, 0:1])
                rstd = smpool.tile([128, 1], f32, tag="rstd")
                nc.vector.reciprocal(rstd[:], std[:])
                nb = smpool.tile([128, 1], f32, tag="nb")
                nc.vector.tensor_scalar(nb[:], mv[:, 0:1], rstd[:, 0:1], -1.0,
                                        OP.mult, OP.mult)
                if trivial:
                    nc.scalar.activation(dst, src_ap,
                                         AF.Relu if relu else AF.Identity,
                                         bias=nb[:, 0:1], scale=rstd[:, 0:1])
                else:
                    z = mlppool.tile([128, width], f32, tag=f"lnz{width}")
                    nc.scalar.activation(z[:], src_ap, AF.Identity,
                                         bias=nb[:, 0:1], scale=rstd[:, 0:1])
                    g_ap = aff_sb[gname][:, lidx, :]
                    b_ap = aff_sb[bname][:, lidx, :]
                    nc.vector.tensor_tensor(z[:], z[:], g_ap, op=OP.mult)
                    if relu:
                        nc.vector.tensor_tensor(z[:], z[:], b_ap, op=OP.add)
                        nc.scalar.activation(dst, z[:], AF.Relu)
                    else:
                        nc.vector.tensor_tensor(dst, z[:], b_ap, op=OP.add)

            # ================= encoder =================
            for w in range(c.NW):
                xt_t = mlppool.tile([128, 128], bf16, tag="xt_enc")
                nc.sync.dma_start(xt_t[:], xT[:, w * 128:(w + 1) * 128])
                h0_ps = psB.tile([128, H], f32, tag="tr")
                nc.tensor.matmul(h0_ps[:], xt_t[:], We_sb[:], start=True, stop=True)
                if trivial:
                    nc.vector.tensor_copy(h_sb[:, w, :], h0_ps[:])
                else:
                    nc.vector.tensor_tensor(h_sb[:, w, :], h0_ps[:],
                                            aff_sb["be"][:], op=OP.add)
                nc.scalar.activation(t_sb[:, w, :], h_sb[:, w, :], AF.Identity)

            rg = [list(range(c.NC))]

            def stage_full():
                nc.sync.dma_start(
                    t_stage[:].rearrange("(pp w) h -> pp (w h)", pp=128),
                    t_sb[:].rearrange("pp w h -> pp (w h)"))

            def allgather(l):
                nc.gpsimd.collective_compute(
                    "AllGather", OP.bypass, replica_groups=rg,
                    ins=[t_stage[:]], outs=[t_fulls[l][:]])

            stage_full()
            allgather(0)

            # memset message pool buffers once (stale-SBUF guard: padding
            # slots are never DMA'd; S zeros them, but NaN*0 would poison PE)
            for par in range(4):
                for q in range(c.NCHUNK):
                    mt = msgpool.tile([128, Tmax, 128], tdt, tag=f"msg{q}")
                    nc.vector.memset(mt[:].rearrange("pp t n -> pp (t n)"), 0.0)

            # ================= conv layers =================
            pool_ps = None
            for l in range(L):
                for g in range(c.NGRP):
                    gw = c.grp_windows(g)
                    gtb = int(p.sec_tile_base[g, 0])
                    ntile_g = grp_tiles[g]
                    smb = grp_smat_base[g]
                    nsm_g = grp_smat[g]
                    s_t = spool.tile([128, GS, c.WIN], bf16, tag="s")
                    nc.sync.dma_start(
                        s_t[:, 0:nsm_g, :],
                        S_d[smb:smb + nsm_g, :, :].rearrange("t pp n -> pp t n"))
                    e_t = epool.tile([128, GT, H], bf16, tag="e")
                    nc.sync.dma_start(
                        e_t[:, 0:ntile_g, :],
                        E_d[gtb:gtb + ntile_g, :, :].rearrange("t pp n -> pp t n"))
                    m_bf = mbfpool.tile([128, GT, 128], bf16, tag="mbf")
                    for q in range(c.NCHUNK):
                        Tq = int(p.T[g, q])
                        nidx = Tq * 128
                        toff = int(p.sec_tile_base[g, q]) - gtb
                        msg = msgpool.tile([128, Tmax, 128], tdt, tag=f"msg{q}")
                        colbase = (gtb + toff) * 8
                        ci = g * c.NCHUNK + q
                        qq = q % nq
                        nc.gpsimd.reg_load(cnt_regs[qq], cnt_sb[0:1, ci:ci + 1])
                        nc.gpsimd.dma_gather(
                            msg[:, 0:Tq, :],
                            t_fulls[l][q * c.CH:(q + 1) * c.CH, :],
                            idx_sb[:, colbase:colbase + nidx // 16],
                            nidx, cnt_regs[qq], elem_size=H, elem_step=H,
                            single_packet=False, queue_num=qq)
                        msl = msg[:, 0:Tq, :].rearrange("pp t n -> pp (t n)")
                        nc.vector.tensor_tensor(
                            msl, msl,
                            e_t[:, toff:toff + Tq, :].rearrange("pp t n -> pp (t n)"),
                            op=OP.add)
                        nc.vector.tensor_scalar(
                            m_bf[:, toff:toff + Tq, :].rearrange("pp t n -> pp (t n)"),
                            msl, 0.0, EPS_MSG, OP.max, OP.add)
                    for wg in range(gw):
                        w = g * c.GW + wg
                        lst = sched[(g, wg)]
                        agg_ps = psA.tile([128, H], f32, tag="agg")
                        if not lst:
                            nc.vector.memset(agg_ps[:], 0.0)
                        else:
                            for j, (q, t, sm) in enumerate(lst):
                                ti = int(p.sec_tile_base[g, q]) - gtb + t
                                nc.tensor.matmul(
                                    agg_ps[:], s_t[:, sm - smb, :],
                                    m_bf[:, ti, :],
                                    start=(j == 0), stop=(j == len(lst) - 1))
                        # ---- window MLP ----
                        X = mlppool.tile([128, H], f32, tag="X")
                        nc.vector.tensor_tensor(X[:], agg_ps[:], t_sb[:, w, :],
                                                op=OP.add)
                        xt_ps = psB.tile([128, 128], f32, tag="tr")
                        nc.tensor.transpose(xt_ps[:], X[:], identf[:])
                        XT = mlppool.tile([128, 128], bf16, tag="XT")
                        nc.scalar.activation(XT[:], xt_ps[:], AF.Identity)
                        y1_ps = psB.tile([128, 2 * H], f32, tag="y")
                        nc.tensor.matmul(y1_ps[:], XT[:], W1_sb[:, l, :],
                                         start=True, stop=True)
                        y1 = mlppool.tile([128, 2 * H], f32, tag="y1sb")
                        if not trivial:
                            nc.vector.tensor_tensor(y1[:], y1_ps[:],
                                                    aff_sb["b1"][:, l, :], op=OP.add)
                        else:
                            nc.scalar.activation(y1[:], y1_ps[:], AF.Identity)
                        z2 = mlppool.tile([128, 2 * H], f32, tag="z2")
                        ln_relu_fused(z2[:], y1[:], "g1", "bb1", l,
                                      relu=True, width=2 * H)
                        z2t = mlppool.tile([128, 2, 128], bf16, tag="z2t")
                        for kk in range(2):
                            zt_ps = psB.tile([128, 128], f32, tag="tr")
                            nc.tensor.transpose(zt_ps[:], z2[:, kk * 128:(kk + 1) * 128],
                                                identf[:])
                            nc.scalar.activation(z2t[:, kk, :], zt_ps[:], AF.Identity)
                        y2_ps = psB.tile([128, H], f32, tag="y")
                        for kk in range(2):
                            nc.tensor.matmul(y2_ps[:], z2t[:, kk, :],
                                             W2_sb[:, l, kk, :],
                                             start=(kk == 0), stop=(kk == 1))
                        hn = mlppool.tile([128, H], f32, tag="hn")
                        if l > 0:
                            nc.vector.tensor_tensor(hn[:], y2_ps[:],
                                                    h_sb[:, w, :], op=OP.add)
                        else:
                            nc.vector.tensor_copy(hn[:], y2_ps[:])
                        if not trivial:
                            nc.vector.tensor_tensor(hn[:], hn[:],
                                                    aff_sb["b2"][:, l, :], op=OP.add)
                        if l < L - 1:
                            nc.scalar.activation(h_sb[:, w, :], hn[:], AF.Identity)
                            ln_relu_fused(t_sb[:, w, :], hn[:], "gn", "bn", l,
                                          relu=True, width=H)
                        else:
                            hf = mlppool.tile([128, H], bf16, tag="hf")
                            ln_relu_fused(hf[:], hn[:], "gn", "bn", l,
                                          relu=False, width=H)
                            Sg = mlppool.tile([128, 128], bf16, tag="Sg")
                            nc.vector.tensor_scalar(Sg[:], iota_sb[:],
                                                    bat_sb[:, w:w + 1], None,
                                                    OP.is_equal)
                            if pool_ps is None:
                                pool_ps = psP.tile([128, H], f32, tag="pool")
                            nc.tensor.matmul(pool_ps[:], Sg[:], hf[:],
                                             start=(w == 0), stop=(w == c.NW - 1),
                                             skip_group_check=True)
                if l < L - 1:
                    stage_full()
                    allgather(l + 1)
            psb = mlppool.tile([128, H], f32, tag="psb")
            nc.vector.tensor_copy(psb[:], pool_ps[:])
            nc.sync.dma_start(out_d[:], psb[:])

    nc.compile()
    return nc


def make_inputs(cfg, inp):
    c = cfg
    import ml_dtypes
    src = np.asarray(inp['edge_index'][0], np.int64)
    dst = np.asarray(inp['edge_index'][1], np.int64)
    attr = np.asarray(inp['edge_attr'], np.int64)
    batch = np.asarray(inp['batch'], np.int64)
    x = np.asarray(inp['x'], np.float32)
    Etab = np.asarray(inp['Etab'], np.float32)
    We = np.asarray(inp['We'], np.float32)
    W1 = np.asarray(inp['W1'], np.float32)
    W2 = np.asarray(inp['W2'], np.float32)

    trivial = (np.all(np.asarray(inp['be']) == 0) and np.all(np.asarray(inp['b1']) == 0)
               and np.all(np.asarray(inp['g1']) == 1) and np.all(np.asarray(inp['bb1']) == 0)
               and np.all(np.asarray(inp['b2']) == 0) and np.all(np.asarray(inp['gn']) == 1)
               and np.all(np.asarray(inp['bn']) == 0))

    bucketed = [bucket_core(c, core, src, dst, attr) for core in range(c.NC)]
    plan, counts = build_plan(c, bucketed)
    sched, nsmat, grp_smat_base = build_smat_schedule(c, plan, bucketed, counts)

    W2s = np.ascontiguousarray(W2.reshape(L, 2, 128, H))
    iota = np.tile(np.arange(128, dtype=np.float32)[None, :], (128, 1))
    bf = ml_dtypes.bfloat16
    in_maps, metas = [], []
    for core in range(c.NC):
        cd = prep_core(c, plan, sched, core, bucketed, batch, Etab)
        xs = x[core * c.SH:(core + 1) * c.SH]
        xTp = np.zeros((128, c.SHP), np.float32)
        xTp[:, :c.SH] = xs.T
        m = {
            'xT': xTp.astype(bf), 'We': We.astype(bf),
            'W1': W1.astype(bf), 'W2': W2s.astype(bf),
            'idx': cd['idx_buf'],
            'S': cd['S'].astype(bf),
            'E': cd['E'].astype(bf),
            'epsdeg': cd['eps_pm'], 'batchrel': cd['batch_pm'],
            'iota': iota,
            'cnt': np.tile(cd['cnts'][None, :], (128, 1)),
        }
        if not trivial:
            rep = lambda v, wdt: np.tile(np.asarray(v, np.float32)[:, None, :], (1, 128, 1))
            m['gn'] = rep(inp['gn'], H); m['bn'] = rep(inp['bn'], H)
            m['g1'] = rep(inp['g1'], 2 * H); m['bb1'] = rep(inp['bb1'], 2 * H)
            m['b1'] = rep(inp['b1'], 2 * H); m['b2'] = rep(inp['b2'], H)
            m['be'] = np.tile(np.asarray(inp['be'], np.float32)[None, :], (128, 1))
        in_maps.append(m)
        metas.append(cd)
    return in_maps, metas, trivial, plan, sched, nsmat, grp_smat_base


def postprocess(cfg, inp, results, metas):
    c = cfg
    batch = np.asarray(inp['batch'], np.int64)
    sums = np.zeros((c.G, H), np.float32)
    for core in range(c.NC):
        part = results[core]['partial']
        g0 = metas[core]['g0']
        b = batch[core * c.SH:(core + 1) * c.SH]
        gmax = int(b.max()) - g0
        sums[g0:g0 + gmax + 1] += part[:gmax + 1]
    cnt = np.bincount(batch, minlength=c.G).astype(np.float32)
    h_graph = sums / np.maximum(cnt, 1.0)[:, None]
    Wp = np.asarray(inp['Wp'], np.float32)
    bp = np.asarray(inp['bp'], np.float32)
    logits = h_graph @ Wp + bp
    return (1.0 / (1.0 + np.exp(-logits))).reshape(-1).astype(np.float32)


_CACHE = {}


def kernel(**inputs):
    from concourse.bass_utils import run_bass_kernel_spmd
    cfg = CFG()
    in_maps, metas, trivial, plan, sched, nsmat, gsb = make_inputs(cfg, inputs)
    key = ('prog', trivial, plan.key)
    if key not in _CACHE:
        _CACHE[key] = build_program(cfg, plan, sched, nsmat, gsb, trivial)
    nc = _CACHE[key]
    res = run_bass_kernel_spmd(nc, in_maps, core_ids=list(range(cfg.NC)))
    return postprocess(cfg, inputs, res.results, metas)

